# revision 29
# baseline (speedup 1.0000x reference)
"""Causal single-head attention on 8 trn2 NeuronCores.

Sharding: batch b = core//2, pair-half h = core%2. Each batch's 4096 queries
split into 4 chunks of 1024; h=0 owns chunks {0,3}, h=1 owns {1,2} (balanced
causal work). Host pre-transposes x into per-(chunk,dc) contiguous tiles in
per-core layout order [ownA, ownB, oth1, oth2] so one uniform SPMD program
runs on all cores; per-core behavior differs only through input data (chunk
order + a tiny per-pass bias table that turns the one never-needed key-chunk
pass off via exp(s - 80)).

Pipeline: xt DMAs issue up front in chunk order; chunk-0 projection
trickles with DMA arrival (warm-up matmuls bridge the gaps so HAM holds
2.4GHz); later chunks' projection matmuls ride as single-matmul filler
units inside pass substeps and in pass tails, with v' transposes
interleaved between real matmuls (transpose-mode looks idle to HAM).
The exp chain runs a 2-deep software pipeline (outT(s) emitted after
scores(s+2), sc psum triple-buffered) so ACT latency stays off the PE
critical path.

Device dataflow (per core, transposed orientation):
  kvt[128, 4096] = [Wk|Wv]^T @ xT        (kT rows 0:64, vT rows 64:128)
  qtt[128, 2048] = (Wq/32)^T @ xT[own]   (gpsimd SBUF DMAs fill row half
                                          64:128 for score row-pairing)
  S^T[keys,q] blocks = kT-slices @ qT    (bf16, causal blocks only; rect
                                          passes pack two K=64 tiles into
                                          PE row groups; diag staircase
                                          tiles trim cols < 128m)
  exp on ACT (PSUM->SBUF) with additive bias; staircase masks via DVE mul
  acc[65, q] += [v|1]^T-tiles @ exp      (row 64 = softmax denominator)
  acc -> DRAM raw; host divides by denominator and transposes.
"""

import sys

if "/opt/trn_rl_repo" not in sys.path:
    sys.path.insert(0, "/opt/trn_rl_repo")

import numpy as np

B, T, D, DK = 4, 4096, 1024, 64
C = 1024          # T-chunk size (4 chunks per batch)
NEG = -80.0       # additive bias for masked chunks: exp(s-80) ~ 1e-35
N_CORES = 8

_CACHE = {}


def _build_nc():
    from contextlib import ExitStack

    import concourse.bass as bass  # noqa: F401
    import concourse.mybir as mybir
    import concourse.tile as tile
    from concourse import bacc
    from concourse.masks import make_identity

    f32 = mybir.dt.float32
    bf16 = mybir.dt.bfloat16
    Exp = mybir.ActivationFunctionType.Exp

    # ---- custom DVE op: out = (((x*C0 + C1)*x + C2)*x + 1)^4 ~= exp(x) ----
    # Splits softmax-exp load between ACT and DVE. Scores here have std
    # ~0.25 (q.k/sqrt(d_model)); the quartic-of-cubic fit is ~1.4e-3 max rel
    # on [-1.5, 1.5], and the scale is exactly 1 so ACT/DVE tiles mix freely
    # in one softmax sum.
    from concourse import dve_ops as _dv
    from concourse.dve_spec import (C0 as _C0, C1 as _C1, C2 as _C2,
                                    One as _One, Spec as _Spec,
                                    Src0 as _S0, sq as _sq)

    def _exp4_ref(in0, in1, s0, s1, imm2):
        x = in0.astype(np.float32)
        t = ((x * s0 + s1) * x + imm2) * x + 1.0
        return (t * t) * (t * t)

    if "EXP4_ANT" not in _dv._SUB_OPCODE_FOR_NAME:
        _exp4 = _dv.DveOp(
            "EXP4_ANT",
            _Spec(
                body=_sq(_sq(((_S0 * _C0 + _C1) * _S0 + _C2) * _S0 + _One)),
                reference=_exp4_ref,
            ),
            subdim=False,
            uops_sha={"v3": "e8cedac265df1391", "v4": "8788bc38416cad3e"},
        )
        _dv.OPS.append(_exp4)
        _dv._SUB_OPCODE_FOR_NAME["EXP4_ANT"] = _dv._CUSTOM_DVE_ROW_BASE + len(_dv.OPS) - 1
        _dv.CUSTOM_DVE_SPECS["EXP4_ANT"] = _exp4.spec
    EXP4 = next(op for op in _dv.OPS if op.name == "EXP4_ANT")
    EA3, EA2, EA1 = 0.002520886, 0.031704176, 0.250250557

    nc = bacc.Bacc("TRN2", target_bir_lowering=False, debug=False,
                   num_devices=N_CORES)

    # xt: per-(chunk,dc) contiguous [128, 1024] tiles, chunk-major in
    # per-core layout order [ownA, ownB, oth1, oth2].
    xt_d = nc.dram_tensor("xt", [4, 4, 128, 2 * C], bf16,
                          kind="ExternalInput").ap()
    # weights pre-permuted on host for contiguous DMA into [128, dc, m] tiles
    wq_d = nc.dram_tensor("wq", [128, 8, DK], bf16, kind="ExternalInput").ap()
    wkv_d = nc.dram_tensor("wkv", [128, 8, 2 * DK], bf16,
                           kind="ExternalInput").ap()
    pb_d = nc.dram_tensor("pbias", [128, 6], f32, kind="ExternalInput").ap()
    # raw accumulators out: per qg [65, 512]: rows 0:64 unnormalized outT,
    # row 64 softmax denominator. Host divides + transposes.
    out_d = nc.dram_tensor("out", [4, DK + 1, 512], f32,
                           kind="ExternalOutput").ap()

    # pass table: (q-chunk sel, key layout position, diag?, bias column)
    # layout per core: [ownA, ownB, oth1, oth2]
    #   h=0: [c0, c3, c1, c2];  h=1: [c1, c2, c0, c3]
    # P2 dead for h=0 (bias col 2 = -80), P4 dead for h=1 (col 4 = -80).
    PASSES = [
        (0, 0, True, None),   # P0: qA vs own chunk A (diagonal)
        (1, 1, True, None),   # P1: qB vs own chunk B (diagonal)
        (0, 2, False, 2),     # P2: qA vs pos2   (h0: dead, h1: q1 x c0)
        (1, 2, False, 3),     # P3: qB vs pos2   (h0: q3 x c1, h1: q2 x c0)
        (1, 3, False, 4),     # P4: qB vs pos3   (h0: q3 x c2, h1: dead)
        (1, 0, False, 5),     # P5: qB vs pos0   (h0: q3 x c0, h1: q2 x c1)
    ]

    with tile.TileContext(nc) as tc, ExitStack() as ctx:
        wpool = ctx.enter_context(tc.tile_pool(name="w", bufs=1))
        xtpool = ctx.enter_context(tc.tile_pool(name="xt", bufs=16))
        kvpool = ctx.enter_context(tc.tile_pool(name="kv", bufs=1))
        exppool = ctx.enter_context(tc.tile_pool(name="exp", bufs=6))
        outsb = ctx.enter_context(tc.tile_pool(name="outsb", bufs=4))
        scps = ctx.enter_context(tc.tile_pool(name="scps", bufs=3, space="PSUM"))
        ppps = ctx.enter_context(tc.tile_pool(name="ppps", bufs=1, space="PSUM"))
        prps = ctx.enter_context(tc.tile_pool(name="prps", bufs=1, space="PSUM"))

        # ---- constants / weights ----
        ident = wpool.tile([128, 128], f32)
        make_identity(nc, ident[:])
        ident_b = wpool.tile([128, 128], bf16)
        nc.vector.tensor_copy(ident_b[:], ident[:])
        wkv_s = wpool.tile([128, 8, 2 * DK], bf16)
        nc.sync.dma_start(wkv_s[:], wkv_d[:])
        wq_s = wpool.tile([128, 8, DK], bf16)
        nc.sync.dma_start(wq_s[:], wq_d[:])
        pb_s = wpool.tile([128, 6], f32)
        nc.sync.dma_start(pb_s[:], pb_d[:])

        # ---- xt DMAs (all issued up front; land chunk by chunk) ----
        xt_tiles = {}
        for lc in range(4):
            for j in range(4):
                t = xtpool.tile([128, 2 * C], bf16, tag="xt",
                                name=f"xt{lc}_{j}")
                nc.sync.dma_start(t[:], xt_d[lc, j])
                xt_tiles[(lc, j)] = t

        # ---- PE warm-up ----
        # No-dependency N=512 matmuls keep the PE busy through the DMA ramp
        # so HAM un-throttles (1.2 -> 2.4 GHz) before the real work arrives.
        wu_sb = wpool.tile([128, 512], bf16)
        nc.vector.memset(wu_sb[:], 0.25)
        wu_ps = scps.tile([128, 1024], f32, tag="sc", name="wu_ps")
        for i in range(6):
            nc.tensor.matmul(
                wu_ps[:, (i % 2) * 512 : (i % 2 + 1) * 512],
                lhsT=wu_sb[:, 0:128], rhs=wu_sb[:],
                start=True, stop=True,
            )

        # ---- persistent activations ----
        kvt = kvpool.tile([128, T], bf16)     # rows 0:64 kT, 64:128 vT
        kdup = kvpool.tile([128, T], bf16)    # rows 64:128 = copy of kT
        qtt = kvpool.tile([128, 2 * C], bf16)  # qT in both row halves
        # inner dim padded to 80 so each kt row is 32B-aligned (DMA xbar)
        vall = kvpool.tile([128, 32, 80], bf16)  # [keys, kt, v|1|pad]
        mten = wpool.tile([128, 4, 512], bf16)   # staircase masks, band m=0..3
        for m in range(4):
            nc.gpsimd.memset(mten[:, m, :], 1.0)
            nc.gpsimd.affine_select(
                out=mten[:, m, :], in_=mten[:, m, :],
                compare_op=mybir.AluOpType.is_ge, fill=0.0,
                base=-(128 * m), channel_multiplier=-1, pattern=[[1, 512]],
            )
        ones_sc = wpool.tile([128, 32], f32)
        nc.vector.memset(ones_sc[:], 1.0)
        nc.vector.tensor_copy(
            vall[:, :, DK : DK + 1],
            ones_sc[:].rearrange("p (a b) -> p a b", b=1),
        )

        # ---- projections ----
        # kv/q projection matmuls are emitted in the tail of the preceding
        # pass (after its last scores, before the flush outTs): the exp of
        # the final substeps runs on ACT while the PE streams proj matmuls.
        # v' transposes interleave between real matmuls (transpose-mode looks
        # idle to HAM, so never emit them as a block).
        def xsl(lc, dc, g):
            return xt_tiles[(lc, dc // 2)][
                :, (dc % 2) * C + g * 512 : (dc % 2) * C + (g + 1) * 512]

        def tr_unit(lc, ktl, dma=False):
            def f():
                kt = lc * 8 + ktl
                if dma:
                    # XBAR transpose on a DMA engine: vT [64,128] -> [128,64]
                    nc.sync.dma_start(
                        vall[:, kt, 0:DK],
                        kvt[64:128, kt * 128 : (kt + 1) * 128],
                        transpose=True,
                    )
                    return
                tr_ps = prps.tile([128, 128], bf16, tag="pr", name="tr_ps")
                nc.tensor.transpose(
                    tr_ps[:],
                    kvt[:, kt * 128 : (kt + 1) * 128],
                    ident_b[:],
                )
                nc.vector.tensor_copy(vall[:, kt, 0:DK], tr_ps[:, 64:128])
            return f

        proj_state = {}

        def kv_mm_unit(lc, g, dc):
            def f():
                cs = slice(lc * C + g * 512, lc * C + (g + 1) * 512)
                if dc == 0:
                    proj_state[(lc, "kv", g)] = prps.tile(
                        [128, 512], f32, tag="pr", name=f"kvu{lc}_{g}")
                kv_ps = proj_state[(lc, "kv", g)]
                nc.tensor.matmul(
                    kv_ps[:],
                    lhsT=wkv_s[:, dc, :],
                    rhs=xsl(lc, dc, g),
                    start=(dc == 0), stop=(dc == 7),
                )
                if dc == 7:
                    nc.vector.tensor_copy(kvt[:, cs], kv_ps[:])
                    del proj_state[(lc, "kv", g)]
                    if g == 1:
                        nc.gpsimd.dma_start(
                            kdup[64:128, lc * C : (lc + 1) * C],
                            kvt[0:64, lc * C : (lc + 1) * C])
            return f

        def q_mm_unit(lc, g, dc):
            def f():
                qc = lc * C
                if dc == 0:
                    proj_state[(lc, "q", g)] = prps.tile(
                        [64, 512], f32, tag="pr", name=f"qu{lc}_{g}")
                q_ps = proj_state[(lc, "q", g)]
                nc.tensor.matmul(
                    q_ps[:],
                    lhsT=wq_s[:, dc, :],
                    rhs=xsl(lc, dc, g),
                    start=(dc == 0), stop=(dc == 7),
                )
                if dc == 7:
                    nc.vector.tensor_copy(
                        qtt[0:64, qc + g * 512 : qc + (g + 1) * 512], q_ps[:])
                    del proj_state[(lc, "q", g)]
                    if g == 1:
                        nc.gpsimd.dma_start(qtt[64:128, qc : qc + C],
                                            qtt[0:64, qc : qc + C])
            return f

        def emit_kv(lc, wu_every=0):
            cc = slice(lc * C, (lc + 1) * C)
            kvq_ps = scps.tile([128, 1024], f32, tag="sc", name=f"kv{lc}")
            n = 0
            for g in range(2):
                for dc in range(8):
                    nc.tensor.matmul(
                        kvq_ps[:, g * 512 : (g + 1) * 512],
                        lhsT=wkv_s[:, dc, :],
                        rhs=xsl(lc, dc, g),
                        start=(dc == 0), stop=(dc == 7),
                    )
                    n += 1
                    if wu_every and n % wu_every == 0:
                        pop_fillers(1)
            nc.vector.tensor_copy(kvt[:, cc], kvq_ps[:])
            nc.gpsimd.dma_start(kdup[64:128, cc], kvt[0:64, cc])

        def emit_q(lc, trs=None):
            # q proj matmuls with this chunk's v' transposes interleaved
            q_ps = scps.tile([64, 1024], f32, tag="sc", name=f"q{lc}")
            trs = list(trs or [])
            for g in range(2):
                for dc in range(8):
                    nc.tensor.matmul(
                        q_ps[:, g * 512 : (g + 1) * 512],
                        lhsT=wq_s[:, dc, :],
                        rhs=xsl(lc, dc, g),
                        start=(dc == 0), stop=(dc == 7),
                    )
                    if dc % 2 == 1 and trs:
                        trs.pop(0)()
            while trs:
                trs.pop(0)()
            qc = lc * C
            nc.vector.tensor_copy(qtt[0:64, qc : qc + C], q_ps[:])
            nc.gpsimd.dma_start(qtt[64:128, qc : qc + C],
                                qtt[0:64, qc : qc + C])

        filler = []        # FIFO of warm-up filler units

        def pop_fillers(n):
            for _ in range(min(n, len(filler))):
                filler.pop(0)()

        def wu_unit(i):
            def f():
                nc.tensor.matmul(
                    wu_ps[:, (i % 2) * 512 : (i % 2 + 1) * 512],
                    lhsT=wu_sb[:, 0:128], rhs=wu_sb[:],
                    start=True, stop=True,
                )
            return f

        # ---- attention passes ----
        # acc[qg] accumulates outT in SBUF (DVE adds of per-pass PSUM
        # partials) so scores PSUM can double-buffer.
        acc = {}       # qg (0..3) -> SBUF accumulator [65, 512]

        def drain(qg):
            nc.sync.dma_start(out_d[qg], acc[qg][:])

        # Flat substep list with a 1-deep software pipeline: scores(s+1) is
        # emitted before outT(s) so the PE never stalls waiting on ACT's exp.
        def make_steps(pi):
            qsel, kp, diag, bcol = PASSES[pi]
            first = pi in (0, 1)  # first pass touching each acc[qg]
            steps = []
            for qg_l in range(2):
                qg = qsel * 2 + qg_l
                qc0 = qsel * 1024 + qg_l * 512
                n_kt = 4 if (diag and qg_l == 0) else 8
                subs = [list(range(s0, min(s0 + 2, n_kt)))
                        for s0 in range(0, n_kt, 2)]
                n_mm = sum(len(s) for s in subs)
                mm0 = 0
                for si, kts in enumerate(subs):
                    steps.append(dict(
                        pi=pi, qg=qg, qc0=qc0, kp=kp, diag=diag, bcol=bcol,
                        qg_l=qg_l, kts=kts, gfirst=(si == 0),
                        glast=(si == len(subs) - 1), mm0=mm0, gmm=n_mm,
                        firstgrp=first,
                        dve=False,
                    ))
                    mm0 += len(kts)
            return steps

        def emit_scores(st):
            kts, w = st["kts"], len(st["kts"])
            qc0 = st["qc0"]
            sc = scps.tile([128, 1024], f32, tag="sc", name="sc")
            # kt pairs go to PE row groups (0,64) and run concurrently.
            # Diag passes run unpaired so they don't depend on the kdup/qtt
            # row-duplicate SBUF DMAs (which queue behind the xt loads).
            for i, ktl in enumerate(kts):
                kc0 = st["kp"] * 1024 + ktl * 128
                half = 0 if st["diag"] else (i % 2) * 64
                lsrc = kvt if half == 0 else kdup
                # staircase tiles: cols < 128m are fully masked; skip them
                # (stale psum there is exp'd then zeroed by the DVE mask)
                m = st["kts"] and (ktl - 4 * st["qg_l"])
                c0 = 128 * m if (st["diag"] and 0 < m < 4) else 0
                nc.tensor.matmul(
                    sc[:, i * 512 + c0 : (i + 1) * 512],
                    lhsT=lsrc[half : half + 64, kc0 : kc0 + 128],
                    rhs=qtt[half : half + 64, qc0 + c0 : qc0 + 512],
                    start=True, stop=True,
                )
            et = exppool.tile([128, 1024], bf16, tag="et", name="et")
            if st["dve"]:
                # P3/P5 carry no bias (cols 3/5 are 0 on both core types)
                nc.vector._custom_dve(
                    EXP4, out=et[:, 0 : w * 512], in0=sc[:, 0 : w * 512],
                    s0=EA3, s1=EA2, imm2=EA1,
                )
            else:
                bcol = st["bcol"]
                bias = pb_s[:, bcol : bcol + 1] if bcol is not None else 0.0
                nc.scalar.activation(et[:, 0 : w * 512], sc[:, 0 : w * 512],
                                     Exp, bias=bias)
            if st["diag"]:
                for i, ktl in enumerate(kts):
                    m = ktl - 4 * st["qg_l"]
                    if 0 <= m < 4:
                        # zero exp where key>q (bf16 DVE mul, off the ACT path)
                        nc.vector.tensor_mul(
                            et[:, i * 512 : (i + 1) * 512],
                            et[:, i * 512 : (i + 1) * 512],
                            mten[:, m, :],
                        )
            st["et"] = et

        pp_of = {}

        def emit_outT(st):
            kts, qg = st["kts"], st["qg"]
            et = st["et"]
            gk = (st["pi"], qg)
            if st["gfirst"]:
                pp_of[gk] = ppps.tile([DK + 1, 512], f32, tag="pp", name="pp")
            pp = pp_of[gk]
            for i, ktl in enumerate(kts):
                kt = st["kp"] * 8 + ktl
                m = ktl - 4 * st["qg_l"]
                c0 = 128 * m if (st["diag"] and 0 < m < 4
                                 and st["mm0"] + i > 0) else 0
                nc.tensor.matmul(
                    pp[:, c0:512],
                    lhsT=vall[:, kt, 0 : DK + 1],
                    rhs=et[:, i * 512 + c0 : (i + 1) * 512],
                    start=(st["mm0"] + i == 0),
                    stop=(st["mm0"] + i == st["gmm"] - 1),
                )
            if st["glast"]:
                if qg not in acc:
                    acc[qg] = outsb.tile([DK + 1, 512], f32, tag="acc",
                                         name=f"acc{qg}")
                if st["firstgrp"]:
                    nc.vector.tensor_copy(acc[qg][:], pp[:])
                else:
                    nc.vector.tensor_add(acc[qg][:], acc[qg][:], pp[:])
                del pp_of[gk]

        pending = []   # up to 2 substeps deep

        def emit_pass(pi, nfill=0, tail=None):
            steps = make_steps(pi)
            for si, st in enumerate(steps):
                emit_scores(st)
                pop_fillers(nfill)
                if si == len(steps) - 1 and tail is not None:
                    tail()   # dense proj matmuls ride the last exp latency
                if len(pending) >= 2:
                    emit_outT(pending.pop(0))
                pending.append(st)

        def flush():
            while pending:
                emit_outT(pending.pop(0))

        def emit_pass_pair(pa, pb):
            # interleave two passes' substeps so pa's exp (DVE) and pb's
            # exp (ACT) run concurrently while the PE streams both
            sa, sb = make_steps(pa), make_steps(pb)
            mix = []
            for i in range(max(len(sa), len(sb))):
                if i < len(sa):
                    mix.append(sa[i])
                if i < len(sb):
                    mix.append(sb[i])
            for st in mix:
                emit_scores(st)
                if len(pending) >= 2:
                    emit_outT(pending.pop(0))
                pending.append(st)

        # ---- schedule ----
        filler.extend(wu_unit(i) for i in range(8))
        emit_kv(0, wu_every=1)       # trickle-gated; warm-ups bridge DMA
        emit_q(0, trs=[tr_unit(0, k) for k in range(8)])
        pop_fillers(99)

        def tail_mix(trs):
            # drain leftover proj units with transposes interleaved 2:1
            # (transpose-mode looks idle to HAM - keep real matmuls flowing)
            def t():
                trs_l = list(trs)
                while filler or trs_l:
                    pop_fillers(2)
                    if trs_l:
                        trs_l.pop(0)()
            return t

        for g in range(2):
            filler.extend(kv_mm_unit(1, g, dc) for dc in range(8))
        for g in range(2):
            filler.extend(q_mm_unit(1, g, dc) for dc in range(8))
        emit_pass(0, nfill=3, tail=tail_mix([tr_unit(1, k) for k in range(8)]))
        flush()
        for g in range(2):
            filler.extend(kv_mm_unit(2, g, dc) for dc in range(8))
        emit_pass(1, nfill=3, tail=tail_mix([tr_unit(2, k) for k in range(8)]))
        flush()
        for g in range(2):
            filler.extend(kv_mm_unit(3, g, dc) for dc in range(8))
        emit_pass(2, nfill=2)        # qA x pos2 (dead for h0)
        emit_pass(3, tail=tail_mix([tr_unit(3, k) for k in range(8)]))
        flush()
        drain(0), drain(1)           # qA accumulators final after P2
        emit_pass(5)                 # qB x pos0 (keys long resident)
        emit_pass(4)                 # qB x pos3 (v' of chunk 3 ready)
        flush()
        drain(2), drain(3)

    nc.compile()
    return nc


def get_nc():
    if "nc" not in _CACHE:
        _CACHE["nc"] = _build_nc()
    return _CACHE["nc"]


def make_in_maps(x, Wq, Wk, Wv):
    import ml_dtypes

    bf = ml_dtypes.bfloat16
    # weights pre-permuted to [128, dc, m] for contiguous DMA
    wq32 = (np.asarray(Wq, np.float32) / 32.0)
    wq_s = np.ascontiguousarray(
        wq32.reshape(8, 128, DK).transpose(1, 0, 2).astype(bf))
    wkv32 = np.concatenate([Wk, Wv], axis=1).astype(np.float32)
    wkv = np.ascontiguousarray(
        wkv32.reshape(8, 128, 2 * DK).transpose(1, 0, 2).astype(bf))
    in_maps = []
    for core in range(N_CORES):
        b, h = core // 2, core % 2
        order = [0, 3, 1, 2] if h == 0 else [1, 2, 0, 3]
        xbt = np.asarray(x[b], np.float32).T  # [D, T] view
        xt = np.empty((4, 4, 128, 2 * C), dtype=bf)
        for li, c in enumerate(order):
            for j in range(4):  # dc pairs -> 4KB DMA lines
                for k in range(2):
                    dc = 2 * j + k
                    xt[li, j, :, k * C : (k + 1) * C] = xbt[
                        dc * 128 : (dc + 1) * 128,
                        c * C : (c + 1) * C].astype(bf)
        bias_vals = [0, 0, NEG, 0, 0, 0] if h == 0 else [0, 0, 0, 0, NEG, 0]
        pb = np.ascontiguousarray(
            np.broadcast_to(np.array(bias_vals, np.float32), (128, 6))
        )
        in_maps.append({"xt": xt, "wq": wq_s, "wkv": wkv, "pbias": pb})
    return in_maps


def gather_out(results):
    out = np.empty((B, T, DK), np.float32)
    for core in range(N_CORES):
        b, h = core // 2, core % 2
        cA, cB = (0, 3) if h == 0 else (1, 2)
        o = results[core]["out"]  # [4, 65, 512]
        for qg in range(4):
            c = cA if qg < 2 else cB
            q0 = c * C + (qg % 2) * 512
            out[b, q0 : q0 + 512] = (o[qg, 0:DK] / o[qg, DK : DK + 1]).T
    return out


def run(in_maps, trace=False, tmpdir=None):
    from concourse.bass_utils import run_bass_kernel_spmd

    nc = get_nc()
    return run_bass_kernel_spmd(
        nc, in_maps, core_ids=list(range(N_CORES)), trace=trace, tmpdir=tmpdir
    )


def kernel(x, Wq, Wk, Wv):
    x = np.asarray(x, dtype=np.float32)
    in_maps = make_in_maps(x, np.asarray(Wq), np.asarray(Wk), np.asarray(Wv))
    res = run(in_maps)
    return gather_out(res.results)
